# revision 30
# baseline (speedup 1.0000x reference)
"""CFConv (gnn message passing) Trainium2 kernel.

Math (per batch b):
    h      = gelu(edge_features @ W1 + b1)        [N, K, C]
    W      = gelu(h @ W2 + b2)                    [N, K, C]
    x_j    = x[b][E_idx[b]]                       [N, K, C]
    out    = sum_k x_j * W                        [N, C]

Sharding: 8 cores = 4 batches x 2 node-halves (2048 nodes / core,
M = 61440 edge rows / core).

Host prep per core (layout only — all FLOPs stay on device):
  - edge rows transposed so the E=300 contraction dim is the SBUF
    partition dim, split into three 100-row chunks e1/e2/e3, cast to
    fp8 e3m4 (4 mantissa bits; halves the HBM traffic this
    memory-bound kernel is limited by vs bf16, and N(0,1) data fits
    the +-15.5 range; e4m3 fails the 2e-2 error gate).  Columns are
    reordered [pair, half, cg, 960] so each unit's 1920 columns are
    one contiguous DMA.
  - xgT [128, M/2] bf16: x[b][E_idx] gathered on host, channel-major,
    group-PAIR stacked (rows 0:64 = even group's 64 channels, 64:128 =
    odd group's) so every op runs at the full 128 partitions.  Kept
    bf16: fp8 here pushes rel-err to the 2e-2 gate.
  - w2dup/b1dup/b2dup duplicated across both partition halves.

Device pipeline: 32 units per core; a unit is half of a 128-stacked
group pair = 2 PSUM subtiles of 480 columns (2 x 16 nodes x 30 k).
  mm1: three accumulating chunk matmuls (contract 100 each, W1
  stationary bf16, moving fp8) with the cg0 chain at PE tile (0,0) and
  the cg1 chain at (0,64) emitted interleaved -> the two 64-wide
  chains execute CONCURRENTLY on disjoint PE column halves.
  gelu(+b1) is ONE fused ScalarE op over both banks [128,1024] ->
  bf16 h -> mm2 (W2 stationary, quadrants (0,0)/(64,64) interleaved)
  -> fused gelu(+b2) -> filter wT -> DVE multiply with streamed x_j^T
  -> DVE groupwise reduce over K=30 -> [128, 32] bf16 -> DMA out.
  mm2/gelu2/DVE of unit u-1 are emitted after mm1 of unit u so the PE
  never stalls on ScalarE, PSUM stays fully double-buffered
  (2+2+2+2 banks), and HAM stays warm.
"""

import os
import sys

import numpy as np

sys.path.insert(0, "/opt/trn_rl_repo")

import ml_dtypes

import concourse.bacc as bacc
import concourse.tile as tile
from concourse import mybir
from concourse.bass_utils import run_bass_kernel_spmd

F32 = mybir.dt.float32
BF16 = mybir.dt.bfloat16
F8E3 = mybir.dt.float8e3
GELU = mybir.ActivationFunctionType.Gelu
BF = ml_dtypes.bfloat16
F8 = ml_dtypes.float8_e3m4

B, N, K, C, E = 4, 4096, 30, 64, 300
NCORES = 8
NPC = N // 2          # nodes per core
M = NPC * K           # edge rows per core = 61440
SUB = 480             # columns per PSUM subtile = 16 nodes x 30 k
UNITS = 32            # units per core; unit = 2 subtiles x 2 cgs
UC = 2 * SUB          # 960 moving columns per cg per unit
NODESU = 2 * UC // K  # 64 output nodes per unit (32 per cg... 2cg x 32)

_CACHE = {}


def build_bass():
    nc = bacc.Bacc(
        "TRN2",
        target_bir_lowering=False,
        debug=False,
        enable_asserts=False,
        num_devices=NCORES,
    )
    # E split (128, 128, 44).  Chunks 1-2 ride a full-128-partition tensor
    # (SDMA engine load balance — 100-partition tiles idle 6 of 16 engines
    # and cap aggregate DMA at ~200 GB/s).  Chunk 3 (44 rows) is split
    # even/odd-unit and lands at SBUF partitions 0-43 / 64-107, whose
    # engine sets complement each other.
    # Chunk 3 is split per-SUBTILE: t0 halves at SBUF partitions 0-43,
    # t1 halves at 64-107, so each unit's four chunk-3 matmuls tile all
    # four PE quadrants and run 4-way concurrent.
    e12 = nc.dram_tensor("e12", [128, UNITS * 2 * 2 * UC], F8E3, kind="ExternalInput").ap()
    e3a = nc.dram_tensor("e3a", [44, UNITS * UC], F8E3, kind="ExternalInput").ap()
    e3b = nc.dram_tensor("e3b", [44, UNITS * UC], F8E3, kind="ExternalInput").ap()
    xgt = nc.dram_tensor("xgt", [128, M // 2], BF16, kind="ExternalInput").ap()
    w1 = nc.dram_tensor("w1", [E, C], BF16, kind="ExternalInput").ap()
    w2d = nc.dram_tensor("w2d", [128, C], BF16, kind="ExternalInput").ap()
    b1d = nc.dram_tensor("b1d", [128, 1], F32, kind="ExternalInput").ap()
    b2d = nc.dram_tensor("b2d", [128, 1], F32, kind="ExternalInput").ap()
    outT = nc.dram_tensor("outT", [128, UNITS * 32], BF16, kind="ExternalOutput").ap()

    with tile.TileContext(nc) as tc:
        with (
            tc.tile_pool(name="const", bufs=1) as pconst,
            tc.tile_pool(name="edge", bufs=4) as pedge,
            tc.tile_pool(name="xjt", bufs=5) as pxjt,
            tc.tile_pool(name="hw", bufs=2) as phw,
            tc.tile_pool(name="mr", bufs=2) as pmr,
            tc.tile_pool(name="ot", bufs=2) as pot,
            tc.tile_pool(name="ps1", bufs=2, space="PSUM") as pps1,
            tc.tile_pool(name="ps2", bufs=2, space="PSUM") as pps2,
        ):
            # 2-unit DMA blocks with a 2-block prefetch lead so the PE
            # never starves; block 0's data is issued before the consts so
            # the pipeline fills as early as possible.
            BU = 2                    # units per DMA block
            NB = UNITS // BU          # 16 blocks
            UB = 2 * 2 * UC           # e12 cols per unit = 3840
            eblk = {}
            e3blk = {}
            xblk = {}

            def issue_block(b, parts=(0, 1, 2)):
                if 0 in parts:
                    et = pedge.tile([128, BU * UB], F8E3, tag="e12")
                    nc.sync.dma_start(et[:], e12[:, b * BU * UB : (b + 1) * BU * UB])
                    eblk[b] = et
                if 1 in parts:
                    e3t = pedge.tile([108, BU * UC], F8E3, tag="e3")
                    s = slice(b * BU * UC, (b + 1) * BU * UC)
                    nc.sync.dma_start(e3t[0:44, :], e3a[:, s])
                    nc.sync.dma_start(e3t[64:108, :], e3b[:, s])
                    e3blk[b] = e3t
                if 2 in parts:
                    xt = pxjt.tile([128, BU * UC], BF16)
                    nc.sync.dma_start(xt[:], xgt[:, b * BU * UC : (b + 1) * BU * UC])
                    xblk[b] = xt

            # DMA issue order tracks first use: e12(b0) and W1 feed the
            # very first matmuls; chunk-3 / biases / xgt come later.
            issue_block(0, parts=(0,))
            w1s = []
            for ci in range(2):
                t = pconst.tile([128, C], BF16, tag=f"w1_{ci}")
                nc.sync.dma_start(t[:], w1[ci * 128 : (ci + 1) * 128, :])
                w1s.append(t)
            # W1[256:300] duplicated at partitions 0-43 and 64-107 so the
            # t1 chunk-3 matmuls can run on PE row-tile 64.
            w1c3 = pconst.tile([108, C], BF16, tag="w1c3")
            nc.sync.dma_start(w1c3[0:44, :], w1[256:E, :])
            nc.sync.dma_start(w1c3[64:108, :], w1[256:E, :])
            b1s = pconst.tile([128, 1], F32, tag="b1s")
            nc.sync.dma_start(b1s[:], b1d)
            issue_block(0, parts=(1,))
            w2s = pconst.tile([128, C], BF16, tag="w2s")
            nc.sync.dma_start(w2s[:], w2d)
            b2s = pconst.tile([128, 1], F32, tag="b2s")
            nc.sync.dma_start(b2s[:], b2d)
            # load the Gelu table set during the pipeline-fill shadow
            warm = pconst.tile([128, 1], BF16, tag="warm")
            nc.scalar.activation(warm[:], b1s[:], GELU)
            issue_block(0, parts=(2,))
            issue_block(1)
            issue_block(2)

            prev = None  # (h2, xgt-block, unit) of unit u-1
            stage = None
            for u in range(UNITS + 1):
                if u < UNITS:
                    if u % BU == 0 and u // BU + 3 < NB:
                        issue_block(u // BU + 3)
                    et = eblk[u // BU]
                    e3t = e3blk[u // BU]
                    xjt = xblk[u // BU]
                    j = u % BU

                    ps1 = pps1.tile([128, 1024], F32)
                    # mm1: cg0 chain at PE tile (0,0), cg1 at (0,64) —
                    # adjacent instructions hit disjoint column halves and
                    # run concurrently.
                    for ci in range(3):
                        for t in range(2):
                            for cg in range(2):
                                po = slice(0, C) if cg == 0 else slice(C, 128)
                                if ci < 2:
                                    base = j * UB + ci * 2 * UC + cg * UC + t * SUB
                                    lhsT = w1s[ci][:]
                                    rhs = et[:, base : base + SUB]
                                    tp = (0, 0) if cg == 0 else (0, C)
                                else:
                                    # 4-way concurrent: t0 on row-tile 0,
                                    # t1 on row-tile 64
                                    base = j * UC + cg * SUB
                                    rp = slice(0, 44) if t == 0 else (slice(64, 108))
                                    ro = 0 if t == 0 else C
                                    lhsT = w1c3[rp, :]
                                    rhs = e3t[rp, base : base + SUB]
                                    tp = (ro, 0) if cg == 0 else (ro, C)
                                nc.tensor.matmul(
                                    ps1[po, t * 512 : t * 512 + SUB],
                                    lhsT,
                                    rhs,
                                    start=(ci == 0),
                                    stop=(ci == 2),
                                    tile_position=tp,
                                    skip_group_check=True,
                                )
                    h2 = phw.tile([128, 1024], BF16, tag="h2")
                    nc.scalar.activation(h2[:, 0:992], ps1[:, 0:992], GELU, bias=b1s[:])
                    cur = (h2, xjt, u)
                if u >= 1:
                    h2v, xjtv, v = prev
                    # mm2 tiles all four PE quadrants (contract 64).  The t1
                    # outputs land on the OPPOSITE psum partition half
                    # (quadrant column = output partitions); the host xgt /
                    # unshard layouts encode the same swap.
                    ps2 = pps2.tile([128, 1024], F32)
                    for t in range(2):
                        for cg in range(2):
                            pin = slice(0, C) if cg == 0 else slice(C, 128)
                            oc = cg ^ t  # output partition half
                            pout = slice(0, C) if oc == 0 else slice(C, 128)
                            nc.tensor.matmul(
                                ps2[pout, t * 512 : t * 512 + SUB],
                                w2s[pin, :],
                                h2v[pin, t * 512 : t * 512 + SUB],
                                start=True,
                                stop=True,
                                tile_position=(cg * C, oc * C),
                                skip_group_check=True,
                            )
                    wt2 = phw.tile([128, 1024], BF16, tag="wt2")
                    nc.scalar.activation(wt2[:, 0:992], ps2[:, 0:992], GELU, bias=b2s[:])
                    mr2 = pmr.tile([128, UC], BF16)
                    xo = (v % BU) * UC
                    nc.vector.tensor_mul(
                        mr2[:, 0:SUB], wt2[:, 0:SUB], xjtv[:, xo : xo + SUB]
                    )
                    nc.vector.tensor_mul(
                        mr2[:, SUB:UC], wt2[:, 512 : 512 + SUB], xjtv[:, xo + SUB : xo + UC]
                    )
                    if v % 8 == 0:
                        stage = pot.tile([128, 8 * 32], BF16)
                    with nc.allow_low_precision(
                        reason="DVE reduce accumulates fp32 internally; "
                        "bf16 is only the final store dtype"
                    ):
                        nc.vector.tensor_reduce(
                            stage[:, (v % 8) * 32 : (v % 8 + 1) * 32],
                            mr2[:].rearrange("p (n k) -> p n k", k=K),
                            axis=mybir.AxisListType.X,
                            op=mybir.AluOpType.add,
                        )
                    if v % 8 == 7:
                        nc.sync.dma_start(
                            outT[:, (v - 7) * 32 : (v + 1) * 32], stage[:]
                        )
                if u < UNITS:
                    prev = cur

    nc.compile()
    return nc


def prep_in_maps(x, edge_features, E_idx, W1, b1, W2, b2):
    x = np.asarray(x, dtype=np.float32)
    edge_features = np.asarray(edge_features, dtype=np.float32)
    E_idx = np.asarray(E_idx)
    W1 = np.asarray(W1, dtype=np.float32)
    b1 = np.asarray(b1, dtype=np.float32)
    W2 = np.asarray(W2, dtype=np.float32)
    b2 = np.asarray(b2, dtype=np.float32)

    shared = {
        "w1": np.ascontiguousarray(W1).astype(BF),
        "w2d": np.ascontiguousarray(np.concatenate([W2, W2], axis=0)).astype(BF),
        "b1d": np.tile(b1.reshape(C, 1), (2, 1)).astype(np.float32),
        "b2d": np.tile(b2.reshape(C, 1), (2, 1)).astype(np.float32),
    }
    in_maps = []
    for c in range(NCORES):
        b = c // 2
        n0 = (c % 2) * NPC
        ef = edge_features[b, n0 : n0 + NPC].reshape(M, E)
        # [E, M] with columns reordered [pair(16), half(2), cg(2), 960]
        # so each unit's 1920 moving columns are contiguous.
        edgeT = ef.T.reshape(E, 16, 2, 2, 960).transpose(0, 1, 3, 2, 4)
        edgeT = np.ascontiguousarray(edgeT.reshape(E, M)).astype(F8)
        # [128, unit(32) x chunk(2) x 1920]: per-partition-contiguous units
        e12 = np.ascontiguousarray(
            edgeT[0:256]
            .reshape(2, 128, UNITS, 2 * UC)
            .transpose(1, 2, 0, 3)
            .reshape(128, UNITS * 2 * 2 * UC)
        )
        # chunk 3 split by subtile: [44, unit x cg x 480] for t0 and t1
        ew = edgeT[256:E].reshape(44, UNITS, 2, 2, SUB)  # [44, u, cg, t, n]
        e3a = np.ascontiguousarray(ew[:, :, :, 0, :].reshape(44, UNITS * UC))
        e3b = np.ascontiguousarray(ew[:, :, :, 1, :].reshape(44, UNITS * UC))
        idx = np.ascontiguousarray(E_idx[b, n0 : n0 + NPC]).reshape(M).astype(np.int64)
        xg = x[b][idx]  # [M, C] f32 host gather
        xjt = np.ascontiguousarray(xg.T)  # [C, M]
        # [128, M/2]: rows (r*64+c), cols (pair*2+half)*960 + t*480 + n,
        # holding cg = r XOR t (the mm2 quadrant map swaps partition
        # halves for the t1 subtile).
        xx6 = xjt.reshape(C, 16, 2, 2, 2, SUB)  # [c, p, cg, h, t, n]
        Y = np.empty((2, C, 16, 2, 2, SUB), dtype=np.float32)  # [r, c, p, h, t, n]
        for r in range(2):
            for t in range(2):
                Y[r, :, :, :, t, :] = xx6[:, :, r ^ t, :, t, :]
        xgt = np.ascontiguousarray(Y.reshape(128, M // 2)).astype(BF)
        in_maps.append(dict(shared, e12=e12, e3a=e3a, e3b=e3b, xgt=xgt))
    return in_maps


def unshard_out(results):
    out = np.empty((B, N, C), dtype=np.float32)
    for c in range(NCORES):
        b = c // 2
        n0 = (c % 2) * NPC
        o = results[c]["outT"].astype(np.float32)
        # rows = (r, ch) holding cg = r XOR t; cols = (pair, half, sub, node16)
        o6 = o.reshape(2, C, 16, 2, 2, 16)  # [r, c, p, h, t, n]
        tgt = np.empty((16, 2, 2, 2, 16, C), dtype=np.float32)  # [p, cg, h, t, n, c]
        o7 = o6.transpose(2, 0, 3, 4, 5, 1)  # [p, r, h, t, n, c]
        for r in range(2):
            for t in range(2):
                tgt[:, r ^ t, :, t] = o7[:, r, :, t]
        out[b, n0 : n0 + NPC] = tgt.reshape(NPC, C)
    return out


def run(in_maps, trace=False):
    if "nc" not in _CACHE:
        _CACHE["nc"] = build_bass()
    nc = _CACHE["nc"]
    kw = {}
    if trace:
        kw["trace"] = True
    res = run_bass_kernel_spmd(nc, in_maps, core_ids=list(range(NCORES)), **kw)
    return res


def kernel(x, edge_features, E_idx, W1, b1, W2, b2):
    in_maps = prep_in_maps(x, edge_features, E_idx, W1, b1, W2, b2)
    res = run(in_maps, trace=bool(os.environ.get("CFCONV_TRACE")))
    if getattr(res, "exec_time_ns", None) is not None:
        print(f"HW exec time: {res.exec_time_ns} ns")
    return unshard_out(res.results)
